# revision 64
# baseline (speedup 1.0000x reference)
"""Trainium2 Bass kernel for nn_AttentionWithVQ (B=4, N=2048, DIM=512, H=8,
depthwise-conv positional term, softmax attention, output projection).

Sharding: data-parallel over B (4 batches x 2 core-groups) and tensor-parallel
over heads (4 heads per core) -> 8 cores, fully independent per core except a
final partial-sum over the two head-groups of each batch, done on host at
gather time (the output projection contracts over heads).

Core algorithmic fusion: the score matrix
    S = 0.5*(scale * q @ k^T + scale * conv1(m) @ conv2(s)^T)
is ONE matmul over a concatenated 128-feature axis:
    S = Qp^T @ Kp,  Qp = [q*scale*0.5 ; conv1(m)*scale*0.5], Kp = [k ; conv2(s)]
which exactly fills the 128x128 PE array contraction dim.

Softmax denominators come for free by appending a ones-column to V
(attn@[V|1] yields the row-sums of exp(S) in the last output row); exp() is
numerically safe without max-subtraction for this problem's score magnitudes.

Schedule: the kernel is paced by the Scalar engine's 128 exp() instructions
(the hard floor at ~1.1us each).  Everything else is arranged around keeping
that stream dense:
  - minimal prologue: only the qkv chunks needed by head 0/1 stripe 0 run
    before the first exp; v-projection, the remaining qkv chunks, the t=1
    convs and the previous stripe's output projection are emitted as PE/DVE
    "fillers" inside the attention nk-loops.
  - loop order stripe-outer/head-inner so each stripe's projection + output
    DMA overlaps the next stripe's attention (no serialized tail).
  - per-(head,stripe) softmax normalization (reciprocal + DRAM-bounce
    partition broadcast) overlapped with the next head's attention.

Partition alignment: compute engines are lane-locked, so per-head feature
layouts alternate by head parity (even heads [qk;conv], odd heads [conv;qk])
making every PSUM->SBUF copy partition-aligned; the few genuinely shifting
copies (odd-head attention outputs, denominator broadcast) go through DMA.
"""


import sys

sys.path.insert(0, "/opt/trn_rl_repo")

import numpy as np

# ---------------------------------------------------------------- constants
B, N, DIM, HEAD, VQE_K = 4, 2048, 512, 8, 3
Dh = DIM // HEAD            # 64
HPC = HEAD // 2             # heads per core (8 cores = 4 batch * 2 groups)
P = 128
NKB = N // P                # 16 key blocks
FB = 512                    # one fp32 PSUM bank
FBS = 1024                  # attention stripe chunk (2 banks)
NST = N // FBS              # 2 q-stripes
SCALE_Q = Dh ** -0.5 * 0.5  # folds the 0.5 score scale into the q/conv1 side

_DEFAULT_CFG = {}
_CACHE = {}


# ---------------------------------------------------------------- host prep
def _host_prep(core, inp):
    """Build the per-core input arrays (sharding + layout permutations)."""
    import ml_dtypes

    bf16 = ml_dtypes.bfloat16
    b, g = core // 2, core % 2
    f32 = np.float32
    x, m, s = inp["x"], inp["m"], inp["s"]
    qkv_w, qkv_b = inp["qkv_w"], inp["qkv_b"]
    proj_w = inp["proj_w"]
    p1w = inp["pe1_w"].reshape(HEAD, VQE_K)
    p2w = inp["pe2_w"].reshape(HEAD, VQE_K)
    pe1_b, pe2_b = inp["pe1_b"], inp["pe2_b"]

    d = {}
    d["xt"] = np.ascontiguousarray(x[b].T).astype(bf16)  # [512, 2048]

    # m/s transposed, tile t rows = [head(2t+1) feats ; head(2t) feats]
    mt = np.empty((256, N), f32)
    st = np.empty((256, N), f32)
    mcw = np.zeros((128, 8), f32)
    scw = np.zeros((128, 8), f32)
    for t in range(2):
        h_lo, h_hi = g * 4 + 2 * t + 1, g * 4 + 2 * t
        mt[t * 128:t * 128 + 64] = m[b][:, h_lo * 64:(h_lo + 1) * 64].T
        mt[t * 128 + 64:t * 128 + 128] = m[b][:, h_hi * 64:(h_hi + 1) * 64].T
        st[t * 128:t * 128 + 64] = s[b][:, h_lo * 64:(h_lo + 1) * 64].T
        st[t * 128 + 64:t * 128 + 128] = s[b][:, h_hi * 64:(h_hi + 1) * 64].T
        for p in range(128):
            h = g * 4 + 2 * t + (1 if p < 64 else 0)
            mcw[p, 4 * t:4 * t + 3] = p1w[h] * SCALE_Q
            scw[p, 4 * t:4 * t + 3] = p2w[h]
            mcw[p, 4 * t + 3] = pe1_b[h] * SCALE_Q
            scw[p, 4 * t + 3] = pe2_b[h]
    d["mt"], d["st"] = mt.astype(bf16), st.astype(bf16)
    d["mcw"], d["scw"] = mcw, scw

    # q/k projection weights: chunk ch=(t, q|k) = [even-head rows; odd-head rows]
    wqk_f = np.empty((512, DIM), f32)
    qkb = np.zeros((128, 4), f32)
    for t in range(2):
        for j in range(2):  # 0=q, 1=k
            ch = 2 * t + j
            h_e, h_o = g * 4 + 2 * t, g * 4 + 2 * t + 1
            base = j * DIM
            wqk_f[ch * 128:ch * 128 + 64] = qkv_w[base + h_e * 64:base + (h_e + 1) * 64]
            wqk_f[ch * 128 + 64:(ch + 1) * 128] = qkv_w[base + h_o * 64:base + (h_o + 1) * 64]
            qkb[0:64, ch] = qkv_b[base + h_e * 64:base + (h_e + 1) * 64]
            qkb[64:128, ch] = qkv_b[base + h_o * 64:base + (h_o + 1) * 64]
            if j == 0:
                wqk_f[ch * 128:(ch + 1) * 128] *= SCALE_Q
                qkb[:, ch] *= SCALE_Q
    d["wqk"] = np.ascontiguousarray(wqk_f.T).astype(bf16)  # [c=512, f=512]
    d["qkb"] = qkb

    d["wv"] = np.ascontiguousarray(
        qkv_w[2 * DIM + g * 256:2 * DIM + (g + 1) * 256].T).astype(bf16)  # [512, 256]
    # v bias replicated along partitions: column order matches wv columns
    vb = qkv_b[2 * DIM + g * 256:2 * DIM + (g + 1) * 256]
    d["vbrep"] = np.broadcast_to(vb, (128, 256)).astype(bf16).copy()

    # proj rows in aT partition order: aT tile t partition p -> head
    # 2t+(p>=64), d=p%64
    pjt = np.empty((256, DIM), f32)
    for t in range(2):
        for p in range(128):
            h_l = 2 * t + (1 if p >= 64 else 0)
            h = g * 4 + h_l
            pjt[t * 128 + p] = proj_w[:, h * 64 + (p % 64)]
    d["pjt"] = pjt.astype(bf16)
    return d


# ------------------------------------------------------------- device build
def _emit(tc, nc, io):
    from contextlib import ExitStack

    from concourse import mybir

    dt = mybir.dt
    f32 = dt.float32
    bf16 = dt.bfloat16
    AF = mybir.ActivationFunctionType
    ALU = mybir.AluOpType

    with ExitStack() as ctx:
        persist = ctx.enter_context(tc.tile_pool(name="persist", bufs=1))
        xtp = ctx.enter_context(tc.tile_pool(name="xtp", bufs=1))
        convp = ctx.enter_context(tc.tile_pool(name="convp", bufs=2))
        convyp = ctx.enter_context(tc.tile_pool(name="convyp", bufs=2))
        # PSUM: s_pool 2x2 banks, o_pool 1x2 banks, shp 2x1 bank = 8 banks
        s_pool = ctx.enter_context(
            tc.tile_pool(name="s_pool", bufs=2, space="PSUM"))
        o_pool = ctx.enter_context(
            tc.tile_pool(name="o_pool", bufs=1, space="PSUM"))
        shp = ctx.enter_context(tc.tile_pool(name="shp", bufs=2, space="PSUM"))
        esb = ctx.enter_context(tc.tile_pool(name="esb", bufs=8))
        stgp = ctx.enter_context(tc.tile_pool(name="stgp", bufs=2))
        denp = ctx.enter_context(tc.tile_pool(name="denp", bufs=2))
        bcp = ctx.enter_context(tc.tile_pool(name="bcp", bufs=2))
        obp = ctx.enter_context(tc.tile_pool(name="obp", bufs=3))

        # ---- persistent tiles
        wqk_sb = [persist.tile([128, 512], bf16, name=f"wqk{c}", tag=f"wqk{c}")
                  for c in range(4)]
        wv_sb = [persist.tile([128, 256], bf16, name=f"wv{c}", tag=f"wv{c}")
                 for c in range(4)]
        pjt_sb = [persist.tile([128, 512], bf16, name=f"pjt{f}", tag=f"pjt{f}")
                  for f in range(2)]
        mcw_sb = persist.tile([128, 8], f32, name="mcw", tag="mcw")
        scw_sb = persist.tile([128, 8], f32, name="scw", tag="scw")
        qkb_sb = persist.tile([128, 4], f32, name="qkb", tag="qkb")
        vbr_sb = persist.tile([128, 256], bf16, name="vbrep", tag="vbrep")
        QP = [persist.tile([128, N], bf16, name=f"QP{h}", tag=f"QP{h}")
              for h in range(HPC)]
        KP = [persist.tile([128, N], bf16, name=f"KP{h}", tag=f"KP{h}")
              for h in range(HPC)]
        # per-head V block is [v(64) | ones | zero-pad] = 66 columns (even
        # width keeps 4-byte operand alignment for bf16)
        v_sb = [persist.tile([128, HPC * 66], bf16, name=f"vsb{b_}",
                             tag=f"vsb{b_}") for b_ in range(NKB)]
        aT = [persist.tile([128, N], bf16, name=f"aT{t}", tag=f"aT{t}")
              for t in range(2)]
        xt_sb = [xtp.tile([128, N], bf16, name=f"xt{c}", tag=f"xt{c}")
                 for c in range(4)]

        # ---- input DMAs, priority order.  Single transfers run at ~23GB/s
        # on one DMA engine, so critical tiles are split into partition
        # halves that run on separate engines concurrently.
        def dma2(q, dst, src, parts=2):
            p = dst.shape[0] // parts
            for i in range(parts):
                q.dma_start(dst[i * p:(i + 1) * p, :], src[i * p:(i + 1) * p, :])

        cin0 = {}
        for src in ("st", "mt"):
            cin0[src] = convp.tile([128, N], bf16, name=f"ci_{src}0",
                                   tag="cin")
        # Only the bytes the first attention iterations need ride in the
        # first wave: stripe-0 columns of xt, the q/k(t0) halves of wqk,
        # the t=0 conv inputs and wv.  Everything else is gated behind the
        # first wave (DMA packets round-robin, so in-flight transfers steal
        # bandwidth from the critical set).
        nc.scalar.dma_start(mcw_sb[:], io["mcw"][:, :])
        nc.scalar.dma_start(scw_sb[:], io["scw"][:, :])
        dma2(nc.sync, cin0["st"][:], io["st"][0:128, :])
        dma2(nc.gpsimd, cin0["mt"][:], io["mt"][0:128, :])
        nc.sync.dma_start(xt_sb[0][:, 0:1024], io["xt"][0:128, 0:1024])
        nc.gpsimd.dma_start(xt_sb[2][:, 0:1024], io["xt"][256:384, 0:1024])
        for c in range(4):
            nc.scalar.dma_start(wqk_sb[c][:, 0:256],
                                io["wqk"][c * 128:(c + 1) * 128, 0:256])
        nc.sync.dma_start(xt_sb[1][:, 0:1024], io["xt"][128:256, 0:1024])
        nc.gpsimd.dma_start(xt_sb[3][:, 0:1024], io["xt"][384:512, 0:1024])
        nc.scalar.dma_start(qkb_sb[:], io["qkb"][:, :])
        for c in range(4):
            nc.gpsimd.dma_start(wv_sb[c][:], io["wv"][c * 128:(c + 1) * 128, :])
        nc.gpsimd.dma_start(vbr_sb[:], io["vbrep"][:, :])


        # ---- helpers -----------------------------------------------------
        convy = {}

        def conv_ops(src, wv_, dst, t, xin, c0=0, c1=N):
            """Depthwise 3-tap conv for columns [c0,c1) of tile t of m/s.
            Column-ranged so the first half of the t=0 convs (which gate the
            first attention iteration) finishes early."""
            key = (src, t)
            if key not in convy:
                convy[key] = convyp.tile([128, N], bf16, name=f"cy_{src}{t}",
                                         tag="cy")
            y = convy[key]
            w0, w1, w2, cb = (wv_[:, 4 * t + k:4 * t + k + 1] for k in range(4))
            lo = max(c0, 1)
            hi = min(c1, N - 1)
            nc.vector.tensor_scalar(y[:, c0:c1], xin[:, c0:c1], w1, cb,
                                    ALU.mult, ALU.add)
            nc.vector.scalar_tensor_tensor(
                y[:, lo:c1], xin[:, lo - 1:c1 - 1], w0, y[:, lo:c1],
                ALU.mult, ALU.add)
            nc.vector.scalar_tensor_tensor(
                y[:, c0:hi], xin[:, c0 + 1:hi + 1], w2, y[:, c0:hi],
                ALU.mult, ALU.add)
            nc.vector.tensor_copy(dst[2 * t + 1][0:64, c0:c1], y[0:64, c0:c1])
            nc.vector.tensor_copy(dst[2 * t][64:128, c0:c1], y[64:128, c0:c1])

        def qkv_chunk(ch, qs, pool, tag, width):
            """q/k projection chunk ch over q-columns qs (width cols)."""
            for step in qkv_chunk_steps(ch, qs, pool, tag, width):
                step()

        def qkv_chunk_steps(ch, qs, pool, tag, width):
            """Same, but as a list of single-matmul emission steps so the
            chunk can be spread across attention iterations."""
            t, j = ch // 2, ch % 2
            dst = QP if j == 0 else KP
            nh = width // FB
            state = {}

            def mk(ih, c):
                def step():
                    if "ps" not in state:
                        state["ps"] = pool.tile([128, width], f32,
                                                name="psqk", tag=tag)
                    ps = state["ps"]
                    nc.tensor.matmul(
                        ps[:, ih * FB:(ih + 1) * FB],
                        wqk_sb[c][:, ch * 128:(ch + 1) * 128],
                        xt_sb[c][:, qs.start + ih * FB:qs.start + (ih + 1) * FB],
                        start=(c == 0), stop=(c == 3))
                    if ih == nh - 1 and c == 3:
                        nc.vector.tensor_scalar_add(
                            dst[2 * t][0:64, qs], ps[0:64, :],
                            qkb_sb[0:64, ch:ch + 1])
                        nc.vector.tensor_scalar_add(
                            dst[2 * t + 1][64:128, qs], ps[64:128, :],
                            qkb_sb[64:128, ch:ch + 1])
                return step

            return [mk(ih, c) for ih in range(nh) for c in range(4)]

        def v_block(blk):
            """v projection for key-block blk + bias + ones/pad columns."""
            bs = slice(blk * 128, (blk + 1) * 128)
            ps = shp.tile([128, 512], f32, name="psv", tag="sh")
            for c in range(4):
                nc.tensor.matmul(ps[:, 0:256], xt_sb[c][:, bs], wv_sb[c][:],
                                 start=(c == 0), stop=(c == 3))
            v3 = v_sb[blk].rearrange("p (h f) -> p h f", h=HPC)
            ps3 = ps.rearrange("p (h f) -> p h f", f=64)
            nc.vector.scalar_tensor_tensor(
                v3[:, :, 0:64], ps3[:, 0:HPC, :],
                1.0, vbr_sb.rearrange("p (h f) -> p h f", h=HPC),
                ALU.mult, ALU.add)
            nc.vector.memset(v3[:, :, 64:65], 1.0)
            nc.vector.memset(v3[:, :, 65:66], 0.0)

        def drain(h, q2, o_ps, mul_eng=None, halves=False, dma_q=None):
            """Release o_ps fast (one staging copy), then normalize by the
            softmax denominators into aT off the critical path.  The
            reciprocal runs 64-partitions-wide via a DMA reshape (a [1,1024]
            single-lane reciprocal costs 6.5us); the multiply defaults to
            the otherwise-idle GpSimd engine.  halves=True pipelines the
            chain in two column halves (used for the tail drain)."""
            t, odd = h // 2, h % 2
            if mul_eng is None:
                mul_eng = nc.gpsimd
            dq = dma_q if dma_q is not None else nc.sync
            row = h * NST + q2
            stg = stgp.tile([65, FBS], bf16, name=f"stg{row}", tag="stg")
            den = denp.tile([64, FBS // 64], bf16, name=f"den{row}", tag="den")
            bc = bcp.tile([64, FBS], bf16, name=f"bc{row}", tag="bc")
            hw_ = FBS // 2 if halves else FBS
            for hs in range(FBS // hw_):
                c0, c1 = hs * hw_, (hs + 1) * hw_
                cs = slice(q2 * FBS + c0, q2 * FBS + c1)
                dn = den[:, hs * hw_ // 64:(hs * hw_ + hw_) // 64]
                # single PSUM read frees o_ps for the next accumulation
                nc.vector.tensor_copy(stg[:, c0:c1], o_ps[0:65, c0:c1])
                dq.dma_start(dn[:], stg[64:65, c0:c1])
                with nc.allow_low_precision(reason="softmax denom fits bf16"):
                    nc.vector.reciprocal(dn[:], dn[:])
                dq.dma_start(io["drec"][row:row + 1, c0:c1], dn[:])
                for i in range(2):
                    dq.dma_start(
                        bc[i * 32:(i + 1) * 32, c0:c1],
                        io["drec"][row:row + 1,
                                   c0:c1].broadcast_to([32, c1 - c0]))
                if odd:
                    mul_eng.tensor_mul(stg[0:64, c0:c1], stg[0:64, c0:c1],
                                       bc[:, c0:c1])
                    for i in range(2):
                        dq.dma_start(
                            aT[t][64 + i * 32:64 + (i + 1) * 32, cs],
                            stg[i * 32:(i + 1) * 32, c0:c1])
                else:
                    mul_eng.tensor_mul(aT[t][0:64, cs], stg[0:64, c0:c1],
                                       bc[:, c0:c1])

        def proj_block_steps(blk):
            """Output projection for 128 q rows + bf16 store, as 2 steps."""
            bs = slice(blk * 128, (blk + 1) * 128)
            state = {}

            def s0():
                state["pj"] = shp.tile([128, FB], f32, name="pj", tag="sh")
                nc.tensor.matmul(state["pj"][:], aT[0][:, bs], pjt_sb[0][:],
                                 start=True, stop=False)

            def s1():
                pj = state["pj"]
                nc.tensor.matmul(pj[:], aT[1][:, bs], pjt_sb[1][:],
                                 start=False, stop=True)
                ob = obp.tile([128, FB], bf16, name="ob", tag="ob")
                nc.vector.tensor_copy(ob[:], pj[:])
                nc.gpsimd.dma_start(io["out"][bs.start:bs.start + 64, :],
                                    ob[0:64, :])
                nc.sync.dma_start(io["out"][bs.start + 64:bs.stop, :],
                                  ob[64:128, :])

            return [s0, s1]

        def proj_block(blk):
            for s in proj_block_steps(blk):
                s()

        # ---- prologue: the k-side t=0 conv (full N: head 0 streams over
        # all key blocks), the first half of the q-side conv, and the two
        # qkv chunks head 0 stripe 0 needs; everything else is emitted as
        # fillers inside the attention loop.
        conv_ops("st", scw_sb, KP, 0, cin0["st"], 0, N)
        conv_ops("mt", mcw_sb, QP, 0, cin0["mt"], 0, 1024)
        # second DMA wave, gated behind the first wave's last arrivals via
        # a tiny Pool op
        gate_sb = persist.tile([1, 16], bf16, name="gate", tag="gate")
        nc.gpsimd.tensor_copy(gate_sb[:], xt_sb[3][0:1, 1000:1016])
        for c in range(4):
            q = (nc.sync, nc.gpsimd)[c % 2]
            q.dma_start(xt_sb[c][:, 1024:2048],
                        io["xt"][c * 128:(c + 1) * 128, 1024:2048])
        for c in range(4):
            nc.scalar.dma_start(wqk_sb[c][:, 256:512],
                                io["wqk"][c * 128:(c + 1) * 128, 256:512])
        # third wave (t=1 conv inputs + proj weights) waits for the second
        # wave's first xt tile so the head-0 filler chunks aren't starved
        gate2_sb = persist.tile([1, 16], bf16, name="gate2", tag="gate2")
        nc.gpsimd.tensor_copy(gate2_sb[:], xt_sb[2][0:1, 2032:2048])
        cin1 = {}
        for src in ("st", "mt"):
            cin1[src] = convp.tile([128, N], bf16, name=f"ci_{src}1",
                                   tag="cin")
            dma2(nc.gpsimd, cin1[src][:], io[src][128:256, :])
        nc.gpsimd.dma_start(pjt_sb[0][:], io["pjt"][0:128, :])
        nc.gpsimd.dma_start(pjt_sb[1][:], io["pjt"][128:256, :])
        # head 0's first score needs only k(t0) cols 0:512; the next 512
        # ride the first filler slots (due by the S(4) emission)
        qkv_chunk(1, slice(0, 512), shp, "sh", 512)
        qkv_chunk(0, slice(0, 1024), s_pool, "sps", 1024)     # q(t0) cols 0:1024

        # ---- attention: stripe-outer, head-inner, exp-paced.  Fillers are
        # single-matmul-sized emission steps, one consumed per nk iteration.
        def fillers_for(h, q2):
            # an entry may be a list of sub-steps (all emitted in one slot)
            fl = []
            if q2 == 0 and h == 0:
                # v blocks ride just-in-time: with the skewed loop, aV(k) is
                # emitted at iter k+1, so v(k) sits at slot k
                for blk in range(NKB):
                    fl.append(lambda b_=blk: v_block(b_))
                # k(t0) cols 1024:2048 in two 512-chunks, due by the S(8)
                # and S(12) emissions; their xt columns arrive with the
                # second DMA wave, so each step rides next to a v block
                qc = qkv_chunk_steps(1, slice(512, 1024), shp, "sh", 512)
                qa = qkv_chunk_steps(1, slice(1024, 1536), shp, "sh", 512)
                qb = qkv_chunk_steps(1, slice(1536, 2048), shp, "sh", 512)
                fl[0] = [fl[0], qc[0], qc[1]]
                fl[1] = [fl[1], qc[2], qc[3]]
                for i, s in enumerate(qa):
                    fl[2 + i] = [fl[2 + i], s]
                for i, s in enumerate(qb):
                    fl[8 + i] = [fl[8 + i], s]
                fl[15] = [fl[15],
                          lambda: conv_ops("st", scw_sb, KP, 1, cin1["st"])]
            elif q2 == 0 and h == 1:
                # k(t1)+q(t1) stripe-0 columns and the t=1 q-side conv
                # (stripe-0 half) — all due by h2 iter 0
                for ch, qb in ((3, 0), (2, 0), (2, 1)):
                    fl += qkv_chunk_steps(ch, slice(qb * 512, (qb + 1) * 512),
                                          shp, "sh", 512)
                fl.insert(6, lambda: conv_ops("mt", mcw_sb, QP, 1,
                                              cin1["mt"], 0, 1024))
                fl.append(lambda: conv_ops("mt", mcw_sb, QP, 0, cin0["mt"],
                                           1024, N))
            elif q2 == 0 and h == 2:
                # k(t1) remaining columns (due by h2 iters 4/8/12) + the
                # stripe-1 half of the t=1 q-side conv
                for ch, qb in ((3, 1), (3, 2), (3, 3)):
                    fl += qkv_chunk_steps(ch, slice(qb * 512, (qb + 1) * 512),
                                          shp, "sh", 512)
                fl.append(lambda: conv_ops("mt", mcw_sb, QP, 1, cin1["mt"],
                                           1024, N))
            elif q2 == 0 and h == 3:
                # q(t0)/q(t1) stripe-1 columns (due by stripe 1)
                for ch, qb in ((0, 2), (0, 3), (2, 2), (2, 3)):
                    fl += qkv_chunk_steps(ch, slice(qb * 512, (qb + 1) * 512),
                                          shp, "sh", 512)
            elif q2 == 1 and h in (1, 3):
                # previous stripe's projection; pad the first slots so the
                # PE never head-of-line blocks on the preceding drain's DMA
                if h == 1:
                    fl += [None] * 6
                for blk in range(2 if h == 1 else 4, 4 if h == 1 else 8):
                    fl += proj_block_steps(blk)
            elif q2 == 1 and h == 0:
                fl += proj_block_steps(0)
                fl += proj_block_steps(1)
            return fl

        for q2 in range(NST):
            for h in ((0, 1, 2, 3) if q2 == 0 else (1, 3, 0, 2)):
                vcols = slice(h * 66, (h + 1) * 66)
                cs0 = q2 * FBS
                fl = fillers_for(h, q2)
                o_full = o_pool.tile([128, FBS], f32, name=f"o{h}_{q2}",
                                     tag="ops")
                o_ps = o_full[0:66, :]
                # skewed pipeline: scores run one iteration ahead of
                # exp/attnV, so the first attnV's wait on the previous
                # head's staging copy hides behind already-queued scores
                def exp_av(nk, s_prev):
                    e = esb.tile([128, FBS], bf16, name="e", tag="e")
                    nc.scalar.activation(e[:], s_prev[:], AF.Exp)
                    for ih in range(2):
                        nc.tensor.matmul(
                            o_ps[:, ih * FB:(ih + 1) * FB], v_sb[nk][:, vcols],
                            e[:, ih * FB:(ih + 1) * FB],
                            start=(nk == 0), stop=(nk == NKB - 1))

                s_prev = None
                for nk in range(NKB):
                    if fl:
                        f = fl.pop(0)
                        for g in (f if isinstance(f, list) else [f]):
                            if g is not None:
                                g()
                    ks = slice(nk * 128, (nk + 1) * 128)
                    s_ps = s_pool.tile([128, FBS], f32, name="sps", tag="sps")
                    for ih in range(2):
                        nc.tensor.matmul(
                            s_ps[:, ih * FB:(ih + 1) * FB], KP[h][:, ks],
                            QP[h][:, cs0 + ih * FB:cs0 + (ih + 1) * FB],
                            start=True, stop=True)
                    if s_prev is not None:
                        exp_av(nk - 1, s_prev)
                    s_prev = s_ps
                exp_av(NKB - 1, s_prev)
                for f in fl:
                    for g in (f if isinstance(f, list) else [f]):
                        if g is not None:
                            g()
                last = q2 == NST - 1 and h == 2
                drain(h, q2, o_ps, mul_eng=nc.vector if last else None,
                      dma_q=nc.scalar if last else None)
            if q2 == NST - 1:
                # last stripe's projection is the tail.  The aT[0]-side
                # matmuls only need the third head's drain, so they run
                # during the last drain's DMA chain (also keeping the PE
                # p-state hot); the aT[1] side + stores follow.  All 8
                # blocks get PSUM slots from the now-idle attention pools.
                slots = []
                for r in range(2):
                    tl = s_pool.tile([128, FBS], f32, name=f"pjs{r}",
                                     tag="sps")
                    slots += [tl[:, 0:FB], tl[:, FB:FBS]]
                tl = o_pool.tile([128, FBS], f32, name="pjo", tag="ops")
                slots += [tl[:, 0:FB], tl[:, FB:FBS]]
                for r in range(2):
                    slots.append(shp.tile([128, FB], f32, name=f"pjh{r}",
                                          tag="sh")[:])
                blks = list(range(q2 * 8, q2 * 8 + 8))
                # three rounds of the aT[0]-side matmuls: the last two are
                # redundant recomputes (start=True overwrites with the same
                # value) that keep the PE busy through the final drain's DMA
                # chain, so it stays at full clock for the aT[1] side
                for _ in range(3):
                    for i, blk in enumerate(blks):
                        bs = slice(blk * 128, (blk + 1) * 128)
                        nc.tensor.matmul(slots[i], aT[0][:, bs], pjt_sb[0][:],
                                         start=True, stop=False,
                                         skip_group_check=True)
                # aT[1]-side matmuls in slot-pair order with one wide cast
                # per PSUM tile right behind each pair, stores trailing
                for p in range(4):
                    for i in (2 * p, 2 * p + 1):
                        bs = slice(blks[i] * 128, (blks[i] + 1) * 128)
                        nc.tensor.matmul(slots[i], aT[1][:, bs], pjt_sb[1][:],
                                         start=False, stop=True)
                    ob = obp.tile([128, FBS], bf16, name=f"ob2_{p}",
                                  tag="ob2")
                    nc.vector.tensor_copy(ob[:, 0:FB], slots[2 * p])
                    nc.vector.tensor_copy(ob[:, FB:FBS], slots[2 * p + 1])
                    for i in (2 * p, 2 * p + 1):
                        bs = slice(blks[i] * 128, (blks[i] + 1) * 128)
                        col = slice(0, FB) if i == 2 * p else slice(FB, FBS)
                        nc.gpsimd.dma_start(
                            io["out"][bs.start:bs.start + 64, :],
                            ob[0:64, col])
                        nc.sync.dma_start(
                            io["out"][bs.start + 64:bs.stop, :],
                            ob[64:128, col])


def _build(cfg_key):
    from concourse import bacc, mybir, tile

    dt = mybir.dt
    nc = bacc.Bacc("TRN2", target_bir_lowering=False, debug=False,
                   num_devices=8)
    shapes = {
        "xt": ([DIM, N], dt.bfloat16),
        "mt": ([256, N], dt.bfloat16), "st": ([256, N], dt.bfloat16),
        "wqk": ([DIM, 512], dt.bfloat16), "wv": ([DIM, 256], dt.bfloat16),
        "pjt": ([256, DIM], dt.bfloat16),
        "mcw": ([128, 8], dt.float32), "scw": ([128, 8], dt.float32),
        "qkb": ([128, 4], dt.float32), "vbrep": ([128, 256], dt.bfloat16),
    }
    io = {}
    for name, (shape, dtt) in shapes.items():
        io[name] = nc.dram_tensor(name, shape, dtt,
                                  kind="ExternalInput").ap()
    io["out"] = nc.dram_tensor("out", [N, DIM], dt.bfloat16,
                               kind="ExternalOutput").ap()
    # internal DRAM bounce for the denominator broadcast (DMA cannot
    # replicate from an SBUF source, but a DRAM source AP is linear and
    # supports a zero-step leading dim)
    io["drec"] = nc.dram_tensor("drec", [HPC * NST, FBS], dt.bfloat16).ap()
    with tile.TileContext(nc) as tc:
        _emit(tc, nc, io)
    nc.compile()
    return nc


def _get_program(cfg=None):
    key = tuple(sorted(cfg.items())) if cfg else ()
    if key not in _CACHE:
        _CACHE[key] = _build(key)
    return _CACHE[key]


# ------------------------------------------------------------------ wrapper
def kernel(_cfg=None, _want_results=False, **inputs):
    from concourse.bass_utils import run_bass_kernel_spmd

    inputs = {k: np.asarray(v, dtype=np.float32) for k, v in inputs.items()}
    nc = _get_program({})
    in_maps = [_host_prep(core, inputs) for core in range(8)]
    res = run_bass_kernel_spmd(nc, in_maps, list(range(8)))

    out = np.empty((B, N, DIM), np.float32)
    pb = inputs["proj_b"]
    for b in range(B):
        out[b] = (res.results[2 * b]["out"].astype(np.float32)
                  + res.results[2 * b + 1]["out"].astype(np.float32) + pb)
    if _want_results:
        return out, res
    return out


# revision 66
# speedup vs baseline: 1.0165x; 1.0165x over previous
"""Trainium2 Bass kernel for nn_AttentionWithVQ (B=4, N=2048, DIM=512, H=8,
depthwise-conv positional term, softmax attention, output projection).

Sharding: data-parallel over B (4 batches x 2 core-groups) and tensor-parallel
over heads (4 heads per core) -> 8 cores, fully independent per core except a
final partial-sum over the two head-groups of each batch, done on host at
gather time (the output projection contracts over heads).

Core algorithmic fusion: the score matrix
    S = 0.5*(scale * q @ k^T + scale * conv1(m) @ conv2(s)^T)
is ONE matmul over a concatenated 128-feature axis:
    S = Qp^T @ Kp,  Qp = [q*scale*0.5 ; conv1(m)*scale*0.5], Kp = [k ; conv2(s)]
which exactly fills the 128x128 PE array contraction dim.

Softmax denominators come for free by appending a ones-column to V
(attn@[V|1] yields the row-sums of exp(S) in the last output row); exp() is
numerically safe without max-subtraction for this problem's score magnitudes.

Schedule: the kernel is paced by the Scalar engine's 128 exp() instructions
(the hard floor at ~1.1us each).  Everything else is arranged around keeping
that stream dense:
  - minimal prologue: only the qkv chunks needed by head 0/1 stripe 0 run
    before the first exp; v-projection, the remaining qkv chunks, the t=1
    convs and the previous stripe's output projection are emitted as PE/DVE
    "fillers" inside the attention nk-loops.
  - loop order stripe-outer/head-inner so each stripe's projection + output
    DMA overlaps the next stripe's attention (no serialized tail).
  - per-(head,stripe) softmax normalization (reciprocal + DRAM-bounce
    partition broadcast) overlapped with the next head's attention.

Partition alignment: compute engines are lane-locked, so per-head feature
layouts alternate by head parity (even heads [qk;conv], odd heads [conv;qk])
making every PSUM->SBUF copy partition-aligned; the few genuinely shifting
copies (odd-head attention outputs, denominator broadcast) go through DMA.
"""


import sys

sys.path.insert(0, "/opt/trn_rl_repo")

import numpy as np

# ---------------------------------------------------------------- constants
B, N, DIM, HEAD, VQE_K = 4, 2048, 512, 8, 3
Dh = DIM // HEAD            # 64
HPC = HEAD // 2             # heads per core (8 cores = 4 batch * 2 groups)
P = 128
NKB = N // P                # 16 key blocks
FB = 512                    # one fp32 PSUM bank
FBS = 1024                  # attention stripe chunk (2 banks)
NST = N // FBS              # 2 q-stripes
SCALE_Q = Dh ** -0.5 * 0.5  # folds the 0.5 score scale into the q/conv1 side

_DEFAULT_CFG = {}
_CACHE = {}


# ---------------------------------------------------------------- host prep
def _host_prep(core, inp):
    """Build the per-core input arrays (sharding + layout permutations)."""
    import ml_dtypes

    bf16 = ml_dtypes.bfloat16
    b, g = core // 2, core % 2
    f32 = np.float32
    x, m, s = inp["x"], inp["m"], inp["s"]
    qkv_w, qkv_b = inp["qkv_w"], inp["qkv_b"]
    proj_w = inp["proj_w"]
    p1w = inp["pe1_w"].reshape(HEAD, VQE_K)
    p2w = inp["pe2_w"].reshape(HEAD, VQE_K)
    pe1_b, pe2_b = inp["pe1_b"], inp["pe2_b"]

    d = {}
    d["xt"] = np.ascontiguousarray(x[b].T).astype(bf16)  # [512, 2048]

    # m/s transposed, tile t rows = [head(2t+1) feats ; head(2t) feats]
    mt = np.empty((256, N), f32)
    st = np.empty((256, N), f32)
    mcw = np.zeros((128, 8), f32)
    scw = np.zeros((128, 8), f32)
    for t in range(2):
        h_lo, h_hi = g * 4 + 2 * t + 1, g * 4 + 2 * t
        mt[t * 128:t * 128 + 64] = m[b][:, h_lo * 64:(h_lo + 1) * 64].T
        mt[t * 128 + 64:t * 128 + 128] = m[b][:, h_hi * 64:(h_hi + 1) * 64].T
        st[t * 128:t * 128 + 64] = s[b][:, h_lo * 64:(h_lo + 1) * 64].T
        st[t * 128 + 64:t * 128 + 128] = s[b][:, h_hi * 64:(h_hi + 1) * 64].T
        for p in range(128):
            h = g * 4 + 2 * t + (1 if p < 64 else 0)
            mcw[p, 4 * t:4 * t + 3] = p1w[h] * SCALE_Q
            scw[p, 4 * t:4 * t + 3] = p2w[h]
            mcw[p, 4 * t + 3] = pe1_b[h] * SCALE_Q
            scw[p, 4 * t + 3] = pe2_b[h]
    d["mt"], d["st"] = mt.astype(bf16), st.astype(bf16)
    d["mcw"], d["scw"] = mcw, scw

    # q/k projection weights: chunk ch=(t, q|k) = [even-head rows; odd-head rows]
    wqk_f = np.empty((512, DIM), f32)
    qkb = np.zeros((128, 4), f32)
    for t in range(2):
        for j in range(2):  # 0=q, 1=k
            ch = 2 * t + j
            h_e, h_o = g * 4 + 2 * t, g * 4 + 2 * t + 1
            base = j * DIM
            wqk_f[ch * 128:ch * 128 + 64] = qkv_w[base + h_e * 64:base + (h_e + 1) * 64]
            wqk_f[ch * 128 + 64:(ch + 1) * 128] = qkv_w[base + h_o * 64:base + (h_o + 1) * 64]
            qkb[0:64, ch] = qkv_b[base + h_e * 64:base + (h_e + 1) * 64]
            qkb[64:128, ch] = qkv_b[base + h_o * 64:base + (h_o + 1) * 64]
            if j == 0:
                wqk_f[ch * 128:(ch + 1) * 128] *= SCALE_Q
                qkb[:, ch] *= SCALE_Q
    d["wqk"] = np.ascontiguousarray(wqk_f.T).astype(bf16)  # [c=512, f=512]
    d["qkb"] = qkb

    d["wv"] = np.ascontiguousarray(
        qkv_w[2 * DIM + g * 256:2 * DIM + (g + 1) * 256].T).astype(bf16)  # [512, 256]
    # v bias replicated along partitions: column order matches wv columns
    vb = qkv_b[2 * DIM + g * 256:2 * DIM + (g + 1) * 256]
    d["vbrep"] = np.broadcast_to(vb, (128, 256)).astype(bf16).copy()

    # proj rows in aT partition order: aT tile t partition p -> head
    # 2t+(p>=64), d=p%64
    pjt = np.empty((256, DIM), f32)
    for t in range(2):
        for p in range(128):
            h_l = 2 * t + (1 if p >= 64 else 0)
            h = g * 4 + h_l
            pjt[t * 128 + p] = proj_w[:, h * 64 + (p % 64)]
    d["pjt"] = pjt.astype(bf16)
    return d


# ------------------------------------------------------------- device build
def _emit(tc, nc, io):
    from contextlib import ExitStack

    from concourse import mybir

    dt = mybir.dt
    f32 = dt.float32
    bf16 = dt.bfloat16
    AF = mybir.ActivationFunctionType
    ALU = mybir.AluOpType

    with ExitStack() as ctx:
        persist = ctx.enter_context(tc.tile_pool(name="persist", bufs=1))
        xtp = ctx.enter_context(tc.tile_pool(name="xtp", bufs=1))
        convp = ctx.enter_context(tc.tile_pool(name="convp", bufs=2))
        convyp = ctx.enter_context(tc.tile_pool(name="convyp", bufs=2))
        # PSUM: s_pool 2x2 banks, o_pool 1x2 banks, shp 2x1 bank = 8 banks
        s_pool = ctx.enter_context(
            tc.tile_pool(name="s_pool", bufs=2, space="PSUM"))
        o_pool = ctx.enter_context(
            tc.tile_pool(name="o_pool", bufs=1, space="PSUM"))
        shp = ctx.enter_context(tc.tile_pool(name="shp", bufs=2, space="PSUM"))
        esb = ctx.enter_context(tc.tile_pool(name="esb", bufs=8))
        stgp = ctx.enter_context(tc.tile_pool(name="stgp", bufs=2))
        denp = ctx.enter_context(tc.tile_pool(name="denp", bufs=2))
        bcp = ctx.enter_context(tc.tile_pool(name="bcp", bufs=2))
        obp = ctx.enter_context(tc.tile_pool(name="obp", bufs=3))

        # ---- persistent tiles
        wqk_sb = [persist.tile([128, 512], bf16, name=f"wqk{c}", tag=f"wqk{c}")
                  for c in range(4)]
        wv_sb = [persist.tile([128, 256], bf16, name=f"wv{c}", tag=f"wv{c}")
                 for c in range(4)]
        pjt_sb = [persist.tile([128, 512], bf16, name=f"pjt{f}", tag=f"pjt{f}")
                  for f in range(2)]
        mcw_sb = persist.tile([128, 8], f32, name="mcw", tag="mcw")
        scw_sb = persist.tile([128, 8], f32, name="scw", tag="scw")
        qkb_sb = persist.tile([128, 4], f32, name="qkb", tag="qkb")
        vbr_sb = persist.tile([128, 256], bf16, name="vbrep", tag="vbrep")
        QP = [persist.tile([128, N], bf16, name=f"QP{h}", tag=f"QP{h}")
              for h in range(HPC)]
        KP = [persist.tile([128, N], bf16, name=f"KP{h}", tag=f"KP{h}")
              for h in range(HPC)]
        # per-head V block is [v(64) | ones | zero-pad] = 66 columns (even
        # width keeps 4-byte operand alignment for bf16)
        v_sb = [persist.tile([128, HPC * 66], bf16, name=f"vsb{b_}",
                             tag=f"vsb{b_}") for b_ in range(NKB)]
        aT = [persist.tile([128, N], bf16, name=f"aT{t}", tag=f"aT{t}")
              for t in range(2)]
        xt_sb = [xtp.tile([128, N], bf16, name=f"xt{c}", tag=f"xt{c}")
                 for c in range(4)]

        # ---- input DMAs, priority order.  Single transfers run at ~23GB/s
        # on one DMA engine, so critical tiles are split into partition
        # halves that run on separate engines concurrently.
        def dma2(q, dst, src, parts=2):
            p = dst.shape[0] // parts
            for i in range(parts):
                q.dma_start(dst[i * p:(i + 1) * p, :], src[i * p:(i + 1) * p, :])

        cin0 = {}
        for src in ("st", "mt"):
            cin0[src] = convp.tile([128, N], bf16, name=f"ci_{src}0",
                                   tag="cin")
        # Only the bytes the first attention iterations need ride in the
        # first wave: stripe-0 columns of xt, the q/k(t0) halves of wqk,
        # the t=0 conv inputs and wv.  Everything else is gated behind the
        # first wave (DMA packets round-robin, so in-flight transfers steal
        # bandwidth from the critical set).
        nc.scalar.dma_start(mcw_sb[:], io["mcw"][:, :])
        nc.scalar.dma_start(scw_sb[:], io["scw"][:, :])
        dma2(nc.sync, cin0["st"][:], io["st"][0:128, :])
        dma2(nc.gpsimd, cin0["mt"][:], io["mt"][0:128, :])
        nc.sync.dma_start(xt_sb[0][:, 0:1024], io["xt"][0:128, 0:1024])
        nc.gpsimd.dma_start(xt_sb[2][:, 0:1024], io["xt"][256:384, 0:1024])
        for c in range(4):
            nc.scalar.dma_start(wqk_sb[c][:, 0:256],
                                io["wqk"][c * 128:(c + 1) * 128, 0:256])
        nc.sync.dma_start(xt_sb[1][:, 0:1024], io["xt"][128:256, 0:1024])
        nc.gpsimd.dma_start(xt_sb[3][:, 0:1024], io["xt"][384:512, 0:1024])
        nc.scalar.dma_start(qkb_sb[:], io["qkb"][:, :])
        for c in range(4):
            nc.gpsimd.dma_start(wv_sb[c][:], io["wv"][c * 128:(c + 1) * 128, :])
        nc.gpsimd.dma_start(vbr_sb[:], io["vbrep"][:, :])


        # ---- helpers -----------------------------------------------------
        convy = {}

        def conv_ops(src, wv_, dst, t, xin, c0=0, c1=N):
            """Depthwise 3-tap conv for columns [c0,c1) of tile t of m/s.
            Column-ranged so the first half of the t=0 convs (which gate the
            first attention iteration) finishes early."""
            key = (src, t)
            if key not in convy:
                convy[key] = convyp.tile([128, N], bf16, name=f"cy_{src}{t}",
                                         tag="cy")
            y = convy[key]
            w0, w1, w2, cb = (wv_[:, 4 * t + k:4 * t + k + 1] for k in range(4))
            lo = max(c0, 1)
            hi = min(c1, N - 1)
            nc.vector.tensor_scalar(y[:, c0:c1], xin[:, c0:c1], w1, cb,
                                    ALU.mult, ALU.add)
            nc.vector.scalar_tensor_tensor(
                y[:, lo:c1], xin[:, lo - 1:c1 - 1], w0, y[:, lo:c1],
                ALU.mult, ALU.add)
            nc.vector.scalar_tensor_tensor(
                y[:, c0:hi], xin[:, c0 + 1:hi + 1], w2, y[:, c0:hi],
                ALU.mult, ALU.add)
            nc.vector.tensor_copy(dst[2 * t + 1][0:64, c0:c1], y[0:64, c0:c1])
            nc.vector.tensor_copy(dst[2 * t][64:128, c0:c1], y[64:128, c0:c1])

        def qkv_chunk(ch, qs, pool, tag, width):
            """q/k projection chunk ch over q-columns qs (width cols)."""
            for step in qkv_chunk_steps(ch, qs, pool, tag, width):
                step()

        def qkv_chunk_steps(ch, qs, pool, tag, width):
            """Same, but as a list of single-matmul emission steps so the
            chunk can be spread across attention iterations."""
            t, j = ch // 2, ch % 2
            dst = QP if j == 0 else KP
            nh = width // FB
            state = {}

            def mk(ih, c):
                def step():
                    if "ps" not in state:
                        state["ps"] = pool.tile([128, width], f32,
                                                name="psqk", tag=tag)
                    ps = state["ps"]
                    nc.tensor.matmul(
                        ps[:, ih * FB:(ih + 1) * FB],
                        wqk_sb[c][:, ch * 128:(ch + 1) * 128],
                        xt_sb[c][:, qs.start + ih * FB:qs.start + (ih + 1) * FB],
                        start=(c == 0), stop=(c == 3))
                    if ih == nh - 1 and c == 3:
                        nc.vector.tensor_scalar_add(
                            dst[2 * t][0:64, qs], ps[0:64, :],
                            qkb_sb[0:64, ch:ch + 1])
                        nc.vector.tensor_scalar_add(
                            dst[2 * t + 1][64:128, qs], ps[64:128, :],
                            qkb_sb[64:128, ch:ch + 1])
                return step

            return [mk(ih, c) for ih in range(nh) for c in range(4)]

        def v_block(blk):
            """v projection for key-block blk + bias + ones/pad columns."""
            bs = slice(blk * 128, (blk + 1) * 128)
            ps = shp.tile([128, 512], f32, name="psv", tag="sh")
            for c in range(4):
                nc.tensor.matmul(ps[:, 0:256], xt_sb[c][:, bs], wv_sb[c][:],
                                 start=(c == 0), stop=(c == 3))
            v3 = v_sb[blk].rearrange("p (h f) -> p h f", h=HPC)
            ps3 = ps.rearrange("p (h f) -> p h f", f=64)
            nc.vector.scalar_tensor_tensor(
                v3[:, :, 0:64], ps3[:, 0:HPC, :],
                1.0, vbr_sb.rearrange("p (h f) -> p h f", h=HPC),
                ALU.mult, ALU.add)
            nc.vector.memset(v3[:, :, 64:65], 1.0)
            nc.vector.memset(v3[:, :, 65:66], 0.0)

        def drain(h, q2, o_ps, mul_eng=None, halves=False, dma_q=None):
            """Release o_ps fast (one staging copy), then normalize by the
            softmax denominators into aT off the critical path.  The
            reciprocal runs 64-partitions-wide via a DMA reshape (a [1,1024]
            single-lane reciprocal costs 6.5us); the multiply defaults to
            the otherwise-idle GpSimd engine.  halves=True pipelines the
            chain in two column halves (used for the tail drain)."""
            t, odd = h // 2, h % 2
            if mul_eng is None:
                mul_eng = nc.gpsimd
            dq = dma_q if dma_q is not None else nc.sync
            row = h * NST + q2
            stg = stgp.tile([65, FBS], bf16, name=f"stg{row}", tag="stg")
            den = denp.tile([64, FBS // 64], bf16, name=f"den{row}", tag="den")
            bc = bcp.tile([64, FBS], bf16, name=f"bc{row}", tag="bc")
            hw_ = FBS // 2 if halves else FBS
            for hs in range(FBS // hw_):
                c0, c1 = hs * hw_, (hs + 1) * hw_
                cs = slice(q2 * FBS + c0, q2 * FBS + c1)
                dn = den[:, hs * hw_ // 64:(hs * hw_ + hw_) // 64]
                # single PSUM read frees o_ps for the next accumulation
                nc.vector.tensor_copy(stg[:, c0:c1], o_ps[0:65, c0:c1])
                dq.dma_start(dn[:], stg[64:65, c0:c1])
                with nc.allow_low_precision(reason="softmax denom fits bf16"):
                    nc.vector.reciprocal(dn[:], dn[:])
                dq.dma_start(io["drec"][row:row + 1, c0:c1], dn[:])
                for i in range(2):
                    dq.dma_start(
                        bc[i * 32:(i + 1) * 32, c0:c1],
                        io["drec"][row:row + 1,
                                   c0:c1].broadcast_to([32, c1 - c0]))
                if odd:
                    mul_eng.tensor_mul(stg[0:64, c0:c1], stg[0:64, c0:c1],
                                       bc[:, c0:c1])
                    for i in range(2):
                        dq.dma_start(
                            aT[t][64 + i * 32:64 + (i + 1) * 32, cs],
                            stg[i * 32:(i + 1) * 32, c0:c1])
                else:
                    mul_eng.tensor_mul(aT[t][0:64, cs], stg[0:64, c0:c1],
                                       bc[:, c0:c1])

        def proj_block_steps(blk):
            """Output projection for 128 q rows + bf16 store, as 2 steps."""
            bs = slice(blk * 128, (blk + 1) * 128)
            state = {}

            def s0():
                state["pj"] = shp.tile([128, FB], f32, name="pj", tag="sh")
                nc.tensor.matmul(state["pj"][:], aT[0][:, bs], pjt_sb[0][:],
                                 start=True, stop=False)

            def s1():
                pj = state["pj"]
                nc.tensor.matmul(pj[:], aT[1][:, bs], pjt_sb[1][:],
                                 start=False, stop=True)
                ob = obp.tile([128, FB], bf16, name="ob", tag="ob")
                nc.vector.tensor_copy(ob[:], pj[:])
                nc.gpsimd.dma_start(io["out"][bs.start:bs.start + 64, :],
                                    ob[0:64, :])
                nc.sync.dma_start(io["out"][bs.start + 64:bs.stop, :],
                                  ob[64:128, :])

            return [s0, s1]

        def proj_block(blk):
            for s in proj_block_steps(blk):
                s()

        # ---- prologue: the k-side t=0 conv (full N: head 0 streams over
        # all key blocks), the first half of the q-side conv, and the two
        # qkv chunks head 0 stripe 0 needs; everything else is emitted as
        # fillers inside the attention loop.
        conv_ops("st", scw_sb, KP, 0, cin0["st"], 0, N)
        conv_ops("mt", mcw_sb, QP, 0, cin0["mt"], 0, 1024)
        # second DMA wave, gated behind the first wave's last arrivals via
        # a tiny Pool op
        gate_sb = persist.tile([1, 16], bf16, name="gate", tag="gate")
        nc.gpsimd.tensor_copy(gate_sb[:], xt_sb[3][0:1, 1000:1016])
        for c in range(4):
            q = (nc.sync, nc.gpsimd)[c % 2]
            q.dma_start(xt_sb[c][:, 1024:2048],
                        io["xt"][c * 128:(c + 1) * 128, 1024:2048])
        for c in range(4):
            nc.scalar.dma_start(wqk_sb[c][:, 256:512],
                                io["wqk"][c * 128:(c + 1) * 128, 256:512])
        # third wave (t=1 conv inputs + proj weights) waits for the second
        # wave's first xt tile so the head-0 filler chunks aren't starved
        gate2_sb = persist.tile([1, 16], bf16, name="gate2", tag="gate2")
        nc.gpsimd.tensor_copy(gate2_sb[:], xt_sb[2][0:1, 2032:2048])
        cin1 = {}
        for src in ("st", "mt"):
            cin1[src] = convp.tile([128, N], bf16, name=f"ci_{src}1",
                                   tag="cin")
            dma2(nc.gpsimd, cin1[src][:], io[src][128:256, :])
        nc.gpsimd.dma_start(pjt_sb[0][:], io["pjt"][0:128, :])
        nc.gpsimd.dma_start(pjt_sb[1][:], io["pjt"][128:256, :])
        qkv_chunk(1, slice(0, 1024), s_pool, "sps", 1024)     # k(t0) cols 0:1024
        qkv_chunk(0, slice(0, 1024), s_pool, "sps", 1024)     # q(t0) cols 0:1024

        # ---- attention: stripe-outer, head-inner, exp-paced.  Fillers are
        # single-matmul-sized emission steps, one consumed per nk iteration.
        def fillers_for(h, q2):
            # an entry may be a list of sub-steps (all emitted in one slot)
            fl = []
            if q2 == 0 and h == 0:
                # v blocks ride just-in-time: with the skewed loop, aV(k) is
                # emitted at iter k+1, so v(k) sits at slot k
                for blk in range(NKB):
                    fl.append(lambda b_=blk: v_block(b_))
                # k(t0) cols 1024:2048 in two 512-chunks, due by the S(8)
                # and S(12) emissions; their xt columns arrive with the
                # second DMA wave, so each step rides next to a v block
                qa = qkv_chunk_steps(1, slice(1024, 1536), shp, "sh", 512)
                qb = qkv_chunk_steps(1, slice(1536, 2048), shp, "sh", 512)
                for i, s in enumerate(qa):
                    fl[2 + i] = [fl[2 + i], s]
                for i, s in enumerate(qb):
                    fl[8 + i] = [fl[8 + i], s]
                fl[15] = [fl[15],
                          lambda: conv_ops("st", scw_sb, KP, 1, cin1["st"])]
            elif q2 == 0 and h == 1:
                # k(t1)+q(t1) stripe-0 columns and the t=1 q-side conv
                # (stripe-0 half) — all due by h2 iter 0
                for ch, qb in ((3, 0), (2, 0), (2, 1)):
                    fl += qkv_chunk_steps(ch, slice(qb * 512, (qb + 1) * 512),
                                          shp, "sh", 512)
                fl.insert(6, lambda: conv_ops("mt", mcw_sb, QP, 1,
                                              cin1["mt"], 0, 1024))
                fl.append(lambda: conv_ops("mt", mcw_sb, QP, 0, cin0["mt"],
                                           1024, N))
            elif q2 == 0 and h == 2:
                # k(t1) remaining columns (due by h2 iters 4/8/12) + the
                # stripe-1 half of the t=1 q-side conv
                for ch, qb in ((3, 1), (3, 2), (3, 3)):
                    fl += qkv_chunk_steps(ch, slice(qb * 512, (qb + 1) * 512),
                                          shp, "sh", 512)
                fl.append(lambda: conv_ops("mt", mcw_sb, QP, 1, cin1["mt"],
                                           1024, N))
            elif q2 == 0 and h == 3:
                # q(t0)/q(t1) stripe-1 columns (due by stripe 1)
                for ch, qb in ((0, 2), (0, 3), (2, 2), (2, 3)):
                    fl += qkv_chunk_steps(ch, slice(qb * 512, (qb + 1) * 512),
                                          shp, "sh", 512)
            elif q2 == 1 and h in (1, 3):
                # previous stripe's projection; pad the first slots so the
                # PE never head-of-line blocks on the preceding drain's DMA
                if h == 1:
                    fl += [None] * 6
                for blk in range(2 if h == 1 else 4, 4 if h == 1 else 8):
                    fl += proj_block_steps(blk)
            elif q2 == 1 and h == 0:
                fl += proj_block_steps(0)
                fl += proj_block_steps(1)
            return fl

        for q2 in range(NST):
            for h in ((0, 1, 2, 3) if q2 == 0 else (1, 3, 0, 2)):
                vcols = slice(h * 66, (h + 1) * 66)
                cs0 = q2 * FBS
                fl = fillers_for(h, q2)
                o_full = o_pool.tile([128, FBS], f32, name=f"o{h}_{q2}",
                                     tag="ops")
                o_ps = o_full[0:66, :]
                # skewed pipeline: scores run one iteration ahead of
                # exp/attnV, so the first attnV's wait on the previous
                # head's staging copy hides behind already-queued scores
                def exp_av(nk, s_prev):
                    e = esb.tile([128, FBS], bf16, name="e", tag="e")
                    nc.scalar.activation(e[:], s_prev[:], AF.Exp)
                    for ih in range(2):
                        nc.tensor.matmul(
                            o_ps[:, ih * FB:(ih + 1) * FB], v_sb[nk][:, vcols],
                            e[:, ih * FB:(ih + 1) * FB],
                            start=(nk == 0), stop=(nk == NKB - 1))

                s_prev = None
                for nk in range(NKB):
                    if fl:
                        f = fl.pop(0)
                        for g in (f if isinstance(f, list) else [f]):
                            if g is not None:
                                g()
                    ks = slice(nk * 128, (nk + 1) * 128)
                    s_ps = s_pool.tile([128, FBS], f32, name="sps", tag="sps")
                    for ih in range(2):
                        nc.tensor.matmul(
                            s_ps[:, ih * FB:(ih + 1) * FB], KP[h][:, ks],
                            QP[h][:, cs0 + ih * FB:cs0 + (ih + 1) * FB],
                            start=True, stop=True)
                    if s_prev is not None:
                        exp_av(nk - 1, s_prev)
                    s_prev = s_ps
                exp_av(NKB - 1, s_prev)
                for f in fl:
                    for g in (f if isinstance(f, list) else [f]):
                        if g is not None:
                            g()
                last = q2 == NST - 1 and h == 2
                drain(h, q2, o_ps, mul_eng=nc.vector if last else None,
                      dma_q=nc.scalar if last else None)
            if q2 == NST - 1:
                # last stripe's projection is the tail.  The aT[0]-side
                # matmuls only need the third head's drain, so they run
                # during the last drain's DMA chain (also keeping the PE
                # p-state hot); the aT[1] side + stores follow.  All 8
                # blocks get PSUM slots from the now-idle attention pools.
                slots = []
                for r in range(2):
                    tl = s_pool.tile([128, FBS], f32, name=f"pjs{r}",
                                     tag="sps")
                    slots += [tl[:, 0:FB], tl[:, FB:FBS]]
                tl = o_pool.tile([128, FBS], f32, name="pjo", tag="ops")
                slots += [tl[:, 0:FB], tl[:, FB:FBS]]
                for r in range(2):
                    slots.append(shp.tile([128, FB], f32, name=f"pjh{r}",
                                          tag="sh")[:])
                blks = list(range(q2 * 8, q2 * 8 + 8))
                # three rounds of the aT[0]-side matmuls: the last two are
                # redundant recomputes (start=True overwrites with the same
                # value) that keep the PE busy through the final drain's DMA
                # chain, so it stays at full clock for the aT[1] side
                for _ in range(3):
                    for i, blk in enumerate(blks):
                        bs = slice(blk * 128, (blk + 1) * 128)
                        nc.tensor.matmul(slots[i], aT[0][:, bs], pjt_sb[0][:],
                                         start=True, stop=False,
                                         skip_group_check=True)
                # aT[1]-side matmuls in slot-pair order with one wide cast
                # per PSUM tile right behind each pair, stores trailing
                for p in range(4):
                    for i in (2 * p, 2 * p + 1):
                        bs = slice(blks[i] * 128, (blks[i] + 1) * 128)
                        nc.tensor.matmul(slots[i], aT[1][:, bs], pjt_sb[1][:],
                                         start=False, stop=True)
                    ob = obp.tile([128, FBS], bf16, name=f"ob2_{p}",
                                  tag="ob2")
                    nc.vector.tensor_copy(ob[:, 0:FB], slots[2 * p])
                    nc.vector.tensor_copy(ob[:, FB:FBS], slots[2 * p + 1])
                    for i in (2 * p, 2 * p + 1):
                        bs = slice(blks[i] * 128, (blks[i] + 1) * 128)
                        col = slice(0, FB) if i == 2 * p else slice(FB, FBS)
                        nc.gpsimd.dma_start(
                            io["out"][bs.start:bs.start + 64, :],
                            ob[0:64, col])
                        nc.sync.dma_start(
                            io["out"][bs.start + 64:bs.stop, :],
                            ob[64:128, col])


def _build(cfg_key):
    from concourse import bacc, mybir, tile

    dt = mybir.dt
    nc = bacc.Bacc("TRN2", target_bir_lowering=False, debug=False,
                   num_devices=8)
    shapes = {
        "xt": ([DIM, N], dt.bfloat16),
        "mt": ([256, N], dt.bfloat16), "st": ([256, N], dt.bfloat16),
        "wqk": ([DIM, 512], dt.bfloat16), "wv": ([DIM, 256], dt.bfloat16),
        "pjt": ([256, DIM], dt.bfloat16),
        "mcw": ([128, 8], dt.float32), "scw": ([128, 8], dt.float32),
        "qkb": ([128, 4], dt.float32), "vbrep": ([128, 256], dt.bfloat16),
    }
    io = {}
    for name, (shape, dtt) in shapes.items():
        io[name] = nc.dram_tensor(name, shape, dtt,
                                  kind="ExternalInput").ap()
    io["out"] = nc.dram_tensor("out", [N, DIM], dt.bfloat16,
                               kind="ExternalOutput").ap()
    # internal DRAM bounce for the denominator broadcast (DMA cannot
    # replicate from an SBUF source, but a DRAM source AP is linear and
    # supports a zero-step leading dim)
    io["drec"] = nc.dram_tensor("drec", [HPC * NST, FBS], dt.bfloat16).ap()
    with tile.TileContext(nc) as tc:
        _emit(tc, nc, io)
    nc.compile()
    return nc


def _get_program(cfg=None):
    key = tuple(sorted(cfg.items())) if cfg else ()
    if key not in _CACHE:
        _CACHE[key] = _build(key)
    return _CACHE[key]


# ------------------------------------------------------------------ wrapper
def kernel(_cfg=None, _want_results=False, **inputs):
    from concourse.bass_utils import run_bass_kernel_spmd

    inputs = {k: np.asarray(v, dtype=np.float32) for k, v in inputs.items()}
    nc = _get_program({})
    in_maps = [_host_prep(core, inputs) for core in range(8)]
    res = run_bass_kernel_spmd(nc, in_maps, list(range(8)))

    out = np.empty((B, N, DIM), np.float32)
    pb = inputs["proj_b"]
    for b in range(B):
        out[b] = (res.results[2 * b]["out"].astype(np.float32)
                  + res.results[2 * b + 1]["out"].astype(np.float32) + pb)
    if _want_results:
        return out, res
    return out
